# revision 72
# baseline (speedup 1.0000x reference)
"""Tensor-parallel MultiHeadAttention (GQA + RMSNorm-KV + RoPE) for 8 trn2 cores.

Sharding: KV head h -> core h (HKV=8); Q heads {2h, 2h+1}; x replicated;
Wo row-sharded; host sums the 8 partial outputs.

Matmuls use fp8e4 DoubleRow with a hi+lo split (3-term products) for the
Q/KV projections, attention scores, and the output projection; P (softmax
weights) and V stay bf16.  Weights are pre-scaled by 64 so the fp8 lo-term
stays out of the subnormal floor: the K/V RMSNorm cancels the scale
automatically, Q is descaled through pre-scaled RoPE tables, and the
output projection's 8*64 factor is divided out when copying y to SBUF
(with the 8 coming from ones=1/8 in the softmax denominator).
All intermediates (qT/kT/v/OT) are SBUF-resident - no DRAM round trips.
"""
import sys
sys.path.insert(0, '/opt/trn_rl_repo')
import numpy as np
import ml_dtypes
import concourse.bass as bass
import concourse.tile as tile
from concourse import mybir
from contextlib import ExitStack

F32 = mybir.dt.float32
F32R = mybir.dt.float32r
BF16 = mybir.dt.bfloat16
F8 = mybir.dt.float8e4
E4 = ml_dtypes.float8_e4m3
AF = mybir.ActivationFunctionType
DR = mybir.MatmulPerfMode.DoubleRow

# Problem constants (full size).
B = 2
S = 2048          # sequence per batch
D = 4096          # model dim
HD = 256          # head dim
DQ = 512          # per-core q width (2 heads)
ROPE_BASE = 10000.0
MASKV = -1e10
EPS = 1e-6
N_CORES = 8
WSCALE = 64.0     # weight pre-scale (power of 2; keeps fp8 lo-terms normal)
OSCALE = 8.0      # attention-output pre-scale via ones = 1/OSCALE


def legalize_waits(nc, max_waits=1):
    """This container's walrus encodes at most one sem-wait per instruction.
    Move extra waits onto same-engine NOPs placed just before (engine FIFO
    order makes that equivalent)."""
    n = 0
    for f in nc.m.functions:
        for blk in f.blocks:
            out = []
            for ins in blk.instructions:
                si = ins.sync_info
                if si is not None and si.on_wait and len(si.on_wait) > max_waits:
                    waits = list(si.on_wait)
                    for w in waits[max_waits:]:
                        nop = mybir.InstNoOp(name=nc.get_next_instruction_name())
                        nop.engine = ins.engine
                        nop.sync_info = mybir.SyncInfo(on_wait=[w], on_update=[])
                        out.append(nop)
                    ins.sync_info = mybir.SyncInfo(
                        on_wait=waits[:max_waits], on_update=list(si.on_update or []))
                    n += 1
                out.append(ins)
            blk.instructions.clear()
            for i in out:
                blk.instructions.append(i)
    return n


def build_bass(b=B, s=S, d=D, legalize=True, phases="ABCD"):
    T = b * s              # total tokens
    NF = d // 128          # contraction tiles
    CH = 128               # phase-A token chunk (x is stationary)
    NCH = T // CH
    TTB = s // 128         # token tiles per batch
    NQB = s // 512         # 512-wide query blocks per batch

    nc = bass.Bass()
    # packed fp8 inputs; i-axis order: x/OT are (hi, lo), weights are (lo, hi)
    xpackd = nc.dram_tensor("xpackd", [NCH, 128, NF, 2, CH], F8,
                            kind="ExternalInput")
    wq8d = nc.dram_tensor("wq8d", [128, NF, 2, DQ], F8, kind="ExternalInput")
    wkv8d = nc.dram_tensor("wkv8d", [128, NF, 2, 512], F8,
                           kind="ExternalInput")
    wo8d = nc.dram_tensor("wo8d", [128, 4, 2, d], F8, kind="ExternalInput")
    cosqd = nc.dram_tensor("cosqd", [128, T], F32, kind="ExternalInput")
    sinqd = nc.dram_tensor("sinqd", [128, T], F32, kind="ExternalInput")
    coskd = nc.dram_tensor("coskd", [T, 128], F32, kind="ExternalInput")
    sinkd = nc.dram_tensor("sinkd", [T, 128], F32, kind="ExternalInput")
    kscd = nc.dram_tensor("kscd", [128, 256], F32, kind="ExternalInput")
    vscd = nc.dram_tensor("vscd", [128, 256], F32, kind="ExternalInput")
    identd = nc.dram_tensor("identd", [128, 128], F32R, kind="ExternalInput")
    onesd = nc.dram_tensor("onesd", [128, 128], BF16, kind="ExternalInput")
    trid = nc.dram_tensor("trid", [128, 128], BF16, kind="ExternalInput")
    y = nc.dram_tensor("y", [T, d], BF16, kind="ExternalOutput")

    with tile.TileContext(nc) as tc, ExitStack() as top:
        # const tiles allocated up front; DMAs emitted later so the first x
        # chunk + weight split win the (serialized) DMA-engine queue
        cp = top.enter_context(tc.tile_pool(name="const", bufs=1))
        zero_b = cp.tile([128, 1], F32)
        nc.vector.memset(zero_b[:], 0.0)
        eps_b = cp.tile([128, 1], F32)
        nc.vector.memset(eps_b[:], EPS)
        ksc = cp.tile([128, 256], F32)
        vsc = cp.tile([128, 256], F32)
        ident = cp.tile([128, 128], F32R)
        ones_sb = cp.tile([128, 128], BF16)
        trib = cp.tile([128, 128], BF16)

        # SBUF-resident intermediates
        ip = top.enter_context(tc.tile_pool(name="interm", bufs=1))
        # qT8[p, head, hd-tile, (lo,hi), T]
        qT8 = ip.tile([128, 2, 2, 2, T], F8)
        # kT8[p, hd-tile, (hi,lo), T]
        kT8 = ip.tile([128, 2, 2, T], F8)
        # v16[p, token-tile, hd]
        v16 = ip.tile([128, NCH, 256], BF16)

        # ---------------- Phase A: projections + norm + rope ----------------
        with ExitStack() as pa:
            wp = pa.enter_context(tc.tile_pool(name="wpool", bufs=1))
            xp = pa.enter_context(tc.tile_pool(name="xpool", bufs=3))
            ep = pa.enter_context(tc.tile_pool(name="aeps", bufs=2))
            pp = pa.enter_context(tc.tile_pool(name="apsum", bufs=2, space="PSUM"))
            tp = pa.enter_context(tc.tile_pool(name="atpsum", bufs=2, space="PSUM"))

            wq_sb = wp.tile([128, NF, 2, DQ], F8)
            wkv_sb = wp.tile([128, NF, 2, 512], F8)
            SPLITS = [0, 2, 12, 22, 32]   # first split tiny: the
            # first chunk's matmuls start after just x0 + 2 f-tiles of weights
            NSPLIT = len(SPLITS) - 1

            x_tiles = {}

            def emit_x_dma(ch):
                x_sb = xp.tile([128, NF, 2, CH], F8, tag="x")
                nc.sync.dma_start(x_sb[:], xpackd[ch])
                x_tiles[ch] = x_sb
                return x_sb

            def alloc_x(ch):
                x_sb = xp.tile([128, NF, 2, CH], F8, tag="x")
                x_tiles[ch] = x_sb
                return x_sb

            def emit_x_piece(x_sb, ch, sp):
                f0, f1 = SPLITS[sp], SPLITS[sp + 1]
                nc.sync.dma_start(x_sb[:, f0:f1], xpackd[ch, :, f0:f1])

            # startup: the first three chunks' x arrives in split-aligned
            # pieces interleaved with the weight splits, so each arriving
            # weight split already has matching x data to compute on
            xs0, xs1, xs2 = alloc_x(0), alloc_x(1), alloc_x(2)
            emit_x_piece(xs0, 0, 0)
            for sp in range(NSPLIT):
                f0, f1 = SPLITS[sp], SPLITS[sp + 1]
                nc.sync.dma_start(wq_sb[:, f0:f1], wq8d[:, f0:f1])
                nc.sync.dma_start(wkv_sb[:, f0:f1], wkv8d[:, f0:f1])
                if sp + 1 < NSPLIT:
                    emit_x_piece(xs0, 0, sp + 1)
                emit_x_piece(xs1, 1, sp)
                if sp >= 1:
                    emit_x_piece(xs2, 2, sp - 1)
                if sp == 0:
                    nc.sync.dma_start(ksc[:], kscd[:])
                    nc.sync.dma_start(vsc[:], vscd[:])
                    nc.sync.dma_start(ident[:], identd[:])
            emit_x_piece(xs2, 2, NSPLIT - 1)

            def emit_kv_split(x_sb, ps_kv, sp):
                # kv: x stationary -> ps_kv[token, width]
                f0, f1 = SPLITS[sp], SPLITS[sp + 1]
                for fp_ in range(f0 // 2, f1 // 2):
                    lhs = x_sb[:, 2 * fp_:2 * fp_ + 2, 0, :]
                    nc.tensor.matmul(ps_kv[:], lhs,
                                     wkv_sb[:, 2 * fp_:2 * fp_ + 2, 1, :],
                                     start=(fp_ == 0), stop=False,
                                     perf_mode=DR)
                for f in range(f0, f1):
                    lhs = x_sb[:, f, :, :]
                    nc.tensor.matmul(ps_kv[:], lhs, wkv_sb[:, f, :, :],
                                     start=False, stop=(f == NF - 1),
                                     perf_mode=DR)

            def emit_q_split(x_sb, ps_q, m, sp):
                # q: WEIGHTS stationary, x moving -> ps_q[:, m, :] =
                # q^T[hd-tile m, tokens] directly (no PE transpose later).
                # start=True zeroes the whole 2KB bank, so each m-chain must
                # run to completion before the next one starts
                f0, f1 = SPLITS[sp], SPLITS[sp + 1]
                msl = slice(m * 128, (m + 1) * 128)
                for fp_ in range(f0 // 2, f1 // 2):
                    nc.tensor.matmul(
                        ps_q[:, m, :],
                        wq_sb[:, 2 * fp_:2 * fp_ + 2, 1, msl],
                        x_sb[:, 2 * fp_:2 * fp_ + 2, 0, :],
                        start=(m == 0 and fp_ == 0), stop=False,
                        perf_mode=DR, skip_group_check=True)
                for f in range(f0, f1):
                    nc.tensor.matmul(
                        ps_q[:, m, :], wq_sb[:, f, :, msl], x_sb[:, f, :, :],
                        start=False, stop=(m == 3 and f == NF - 1),
                        perf_mode=DR, skip_group_check=True)

            def get_x(ch):
                x_sb = x_tiles.pop(ch) if ch in x_tiles else emit_x_dma(ch)
                x_tiles.pop(ch, None)
                return x_sb

            def emit_chunk_mms(ch):
                x_sb = get_x(ch)
                ps_q = pp.tile([128, 4, 128], F32, tag="psq")
                ps_kv = pp.tile([128, 512], F32, tag="pskv")
                # grouped by weight DMA split so the first chunks can start
                # before all weight splits have landed
                for sp in range(NSPLIT):
                    emit_kv_split(x_sb, ps_kv, sp)
                for m in range(4):
                    for sp in range(NSPLIT):
                        emit_q_split(x_sb, ps_q, m, sp)
                return ps_q, ps_kv

            def emit_first_two_chunk_mms():
                # interleave chunks 0 and 1 split-by-split: each arriving
                # weight split immediately has two chunks of ready PE work
                xs = [get_x(0), get_x(1)]
                pss = []
                for c in range(2):
                    ps_q = pp.tile([128, 4, 128], F32, tag="psq")
                    ps_kv = pp.tile([128, 512], F32, tag="pskv")
                    pss.append((ps_q, ps_kv))
                for sp in range(NSPLIT):
                    for c in range(2):
                        emit_kv_split(xs[c], pss[c][1], sp)
                for m in range(4):
                    for sp in range(NSPLIT):
                        for c in range(2):
                            emit_q_split(xs[c], pss[c][0], m, sp)
                return pss

            def emit_chunk_dve(ch, ps_q, ps_kv):
                tg = ch * CH
                cosq_t = ep.tile([128, 128], F32, tag="cosq")
                nc.sync.dma_start(cosq_t[:], cosqd[:, tg:tg + 128])
                sinq_t = ep.tile([128, 128], F32, tag="sinq")
                nc.sync.dma_start(sinq_t[:], sinqd[:, tg:tg + 128])
                cosk_t = ep.tile([128, 128], F32, tag="cosk")
                nc.sync.dma_start(cosk_t[:], coskd[tg:tg + 128, :])
                sink_t = ep.tile([128, 128], F32, tag="sink")
                nc.sync.dma_start(sink_t[:], sinkd[tg:tg + 128, :])

                # Q rope in q^T layout (cos/sin tables are [hd-part, T]
                # with the rope frequency = partition); cosq/sinq carry the
                # 1/WSCALE descale
                stage_q = ep.tile([128, 2, 2, 128], F32, tag="stq")
                tmp1 = ep.tile([128, 128], F32, tag="tmp1")
                tmp2 = ep.tile([128, 128], F32, tag="tmp2")
                for h in range(2):
                    fi = ps_q[:, 2 * h, :]
                    se = ps_q[:, 2 * h + 1, :]
                    nc.vector.tensor_mul(tmp1[:], fi, cosq_t[:])
                    nc.vector.tensor_mul(tmp2[:], se, sinq_t[:])
                    nc.vector.tensor_sub(stage_q[:, h, 0, :],
                                         tmp1[:], tmp2[:])
                    nc.vector.tensor_mul(tmp1[:], se, cosq_t[:])
                    nc.vector.tensor_mul(tmp2[:], fi, sinq_t[:])
                    nc.vector.tensor_add(stage_q[:, h, 1, :],
                                         tmp1[:], tmp2[:])
                # split q into qT8 right here -- already transposed
                qhi = qT8[:, :, :, 1, tg:tg + 128]
                nc.vector.tensor_copy(qhi, stage_q[:])
                qhif = ep.tile([128, 2, 2, 128], F32, tag="qhif")
                nc.vector.tensor_copy(qhif[:], qhi)
                nc.vector.tensor_sub(qT8[:, :, :, 0, tg:tg + 128],
                                     stage_q[:], qhif[:])

                # K: rmsnorm + scale + rope (scale-invariant in WSCALE)
                sq = ep.tile([128, 256], F32, tag="sq")
                ssq = ep.tile([128, 1], F32, tag="ssq")
                nc.scalar.activation(sq[:], ps_kv[:, 0:256], AF.Square,
                                     bias=zero_b[:], accum_out=ssq[:])
                std = ep.tile([128, 1], F32, tag="std")
                nc.scalar.activation(std[:], ssq[:], AF.Sqrt,
                                     bias=eps_b[:], scale=1.0 / 256.0)
                rstd = ep.tile([128, 1], F32, tag="rstd")
                nc.vector.reciprocal(rstd[:], std[:])
                kn = ep.tile([128, 256], F32, tag="kn")
                nc.vector.tensor_scalar_mul(kn[:], ps_kv[:, 0:256], rstd[:])
                kn2 = ep.tile([128, 256], F32, tag="kn2")
                nc.vector.tensor_mul(kn2[:], kn[:], ksc[:])
                stage_k = ep.tile([128, 256], F32R, tag="stk")
                nc.vector.tensor_mul(tmp1[:], kn2[:, 0:128], cosk_t[:])
                nc.vector.tensor_mul(tmp2[:], kn2[:, 128:256], sink_t[:])
                nc.vector.tensor_sub(stage_k[:, 0:128], tmp1[:], tmp2[:])
                nc.vector.tensor_mul(tmp1[:], kn2[:, 128:256], cosk_t[:])
                nc.vector.tensor_mul(tmp2[:], kn2[:, 0:128], sink_t[:])
                nc.vector.tensor_add(stage_k[:, 128:256], tmp1[:], tmp2[:])

                # V: rmsnorm + scale -> straight into SBUF-resident v16 (bf16)
                sqv = ep.tile([128, 256], F32, tag="sqv")
                ssqv = ep.tile([128, 1], F32, tag="ssqv")
                nc.scalar.activation(sqv[:], ps_kv[:, 256:512], AF.Square,
                                     bias=zero_b[:], accum_out=ssqv[:])
                stdv = ep.tile([128, 1], F32, tag="stdv")
                nc.scalar.activation(stdv[:], ssqv[:], AF.Sqrt,
                                     bias=eps_b[:], scale=1.0 / 256.0)
                rstdv = ep.tile([128, 1], F32, tag="rstdv")
                nc.vector.reciprocal(rstdv[:], stdv[:])
                vn = ep.tile([128, 256], F32, tag="vn")
                nc.vector.tensor_scalar_mul(vn[:], ps_kv[:, 256:512], rstdv[:])
                nc.vector.tensor_mul(v16[:, ch, :], vn[:], vsc[:])
                return stage_k

            def emit_chunk_transposes(ch, stage_k):
                tg = ch * CH
                tk_ps = tp.tile([128, 256], F32, tag="tk")
                for dblk in range(2):
                    nc.tensor.matmul(
                        tk_ps[:, dblk * 128:(dblk + 1) * 128].bitcast(F32R),
                        stage_k[:, dblk * 128:(dblk + 1) * 128],
                        ident[:], is_transpose=True)
                khi = kT8[:, :, 0, tg:tg + 128]
                nc.vector.tensor_copy(khi, tk_ps[:])
                khif = ep.tile([128, 256], F32, tag="khif")
                nc.vector.tensor_copy(khif[:], khi)
                nc.vector.tensor_sub(kT8[:, :, 1, tg:tg + 128],
                                     tk_ps[:], khif[:])

            pending = None
            for ch in (range(NCH) if "A" in phases else []):
                ps_q, ps_kv = emit_chunk_mms(ch)
                stage_k = emit_chunk_dve(ch, ps_q, ps_kv)
                if pending is not None:
                    emit_chunk_transposes(pending[0], pending[1])
                pending = (ch, stage_k)
            if pending is not None:
                emit_chunk_transposes(pending[0], pending[1])

        # ---------------- Phases B/C: attention ----------------
        with ExitStack() as pot:
            otp = pot.enter_context(tc.tile_pool(name="otpool", bufs=1))
            # OT8[p, g=(head,hd-half), (hi,lo), T], pre-scaled by OSCALE
            OT8 = otp.tile([128, 4, 2, T], F8)
            wo_sb = otp.tile([128, 4, 2, d], F8)
            nc.sync.dma_start(ones_sb[:], onesd[:])
            nc.sync.dma_start(trib[:], trid[:])
            for g in range(4):
                nc.sync.dma_start(wo_sb[:, g], wo8d[:, g])

            if "B" in phases and "C" in phases:
                with ExitStack() as pc:
                    spool = pc.enter_context(
                        tc.tile_pool(name="spsum", bufs=3, space="PSUM"))
                    opool = pc.enter_context(
                        tc.tile_pool(name="opsum", bufs=2, space="PSUM"))
                    rpool = pc.enter_context(
                        tc.tile_pool(name="rpsum", bufs=1, space="PSUM"))
                    ptp = pc.enter_context(tc.tile_pool(name="ptpool", bufs=12))
                    rcp = pc.enter_context(tc.tile_pool(name="rcpool", bufs=2))

                    # unit = ("full", bb, h, tqb, j): unmasked [128k x 512q]
                    # unit = ("diag", bb, h, tqb, qs, jd): [128k x 128q] on
                    #   the causal diagonal; masked triangularly when
                    #   jd == 4*tqb + qs, else unmasked
                    units = []
                    for bb in range(b):
                        for h in range(2):
                            for tqb in range(NQB):
                                for j in range(4 * tqb):
                                    units.append(("full", bb, h, tqb, j))
                                for qs in range(4):
                                    for jd in range(4 * tqb,
                                                    4 * tqb + qs + 1):
                                        units.append(
                                            ("diag", bb, h, tqb, qs, jd))

                    # diag score blocks pack 4-per-PSUM-tile (column slots) so
                    # a deep lookahead fits in 3 score banks
                    dstate = {"tile": None, "slot": 3}

                    def emit_s(u):
                        kind, bb, h, tqb = u[:4]
                        bT = bb * s
                        if kind == "full":
                            j = u[4]
                            qsl = slice(bT + tqb * 512, bT + (tqb + 1) * 512)
                            jsl = slice(bT + j * 128, bT + (j + 1) * 128)
                            s_ps = spool.tile([128, 512], F32, tag="s",
                                              name="s_ps")
                            out = s_ps[:]
                            dstate["tile"] = None
                        else:
                            qs, jd = u[4], u[5]
                            q0 = bT + tqb * 512 + qs * 128
                            qsl = slice(q0, q0 + 128)
                            jsl = slice(bT + jd * 128, bT + (jd + 1) * 128)
                            slot = dstate["slot"] + 1
                            if slot > 3 or dstate["tile"] is None:
                                dstate["tile"] = spool.tile(
                                    [128, 512], F32, tag="s", name="s_ps")
                                slot = 0
                            dstate["slot"] = slot
                            out = dstate["tile"][:, slot * 128:slot * 128 + 128]
                        nc.tensor.matmul(
                            out, kT8[:, :, 0, jsl], qT8[:, h, :, 1, qsl],
                            start=True, stop=False, perf_mode=DR)
                        nc.tensor.matmul(
                            out, kT8[:, 0, :, jsl], qT8[:, h, 0, :, qsl],
                            start=False, stop=False, perf_mode=DR)
                        nc.tensor.matmul(
                            out, kT8[:, 1, :, jsl], qT8[:, h, 1, :, qsl],
                            start=False, stop=True, perf_mode=DR)
                        return out

                    # one software pipeline over all units with a deep score
                    # lookahead: the PE never waits on the scalar engine
                    # across block edges
                    LOOK = 12
                    o_ps0 = o_ps1 = rb_ps = None
                    s_q = [emit_s(uu) for uu in units[:LOOK]]
                    prev_blk = None
                    for i, u in enumerate(units):
                        kind, bb, h, tqb = u[:4]
                        bT = bb * s
                        blk = (bb, h, tqb)
                        s_cur = s_q.pop(0)
                        if kind == "full":
                            pT = ptp.tile([128, 512], BF16, tag="pf")
                            nc.scalar.activation(pT[:], s_cur, AF.Exp,
                                                 bias=zero_b[:], scale=0.0625)
                        else:
                            pT = ptp.tile([128, 128], BF16, tag="pd")
                            nc.scalar.activation(pT[:], s_cur,
                                                 AF.Exp, bias=zero_b[:],
                                                 scale=0.0625)
                            if u[5] == 4 * tqb + u[4]:
                                # causal mask: zero the upper triangle of the
                                # diagonal block on the otherwise-idle Pool
                                # engine (exp of unmasked scores is finite)
                                nc.gpsimd.tensor_mul(pT[:], pT[:], trib[:])
                        if i + LOOK < len(units):
                            s_q.append(emit_s(units[i + LOOK]))
                        if blk != prev_blk:
                            o_ps0 = opool.tile([128, 512], F32, tag="o0")
                            o_ps1 = opool.tile([128, 512], F32, tag="o1")
                            rb_ps = rpool.tile([128, 512], F32, tag="rb")
                            prev_blk = blk
                        if kind == "full":
                            j = u[4]
                            vt = bb * TTB + j
                            st = (j == 0)
                            nc.tensor.matmul(rb_ps[:], ones_sb[:], pT[:],
                                             start=st, stop=False,
                                             skip_group_check=True)
                            nc.tensor.matmul(o_ps0[:], v16[:, vt, 0:128],
                                             pT[:], start=st, stop=False,
                                             skip_group_check=True)
                            nc.tensor.matmul(o_ps1[:], v16[:, vt, 128:256],
                                             pT[:], start=st, stop=False,
                                             skip_group_check=True)
                        else:
                            qs, jd = u[4], u[5]
                            vt = bb * TTB + jd
                            cols = slice(qs * 128, (qs + 1) * 128)
                            st = (tqb == 0 and jd == 0)
                            sp_ = (jd == 4 * tqb + qs)
                            nc.tensor.matmul(rb_ps[:, cols], ones_sb[:],
                                             pT[:], start=st, stop=sp_,
                                             skip_group_check=True)
                            nc.tensor.matmul(o_ps0[:, cols],
                                             v16[:, vt, 0:128], pT[:],
                                             start=st, stop=sp_,
                                             skip_group_check=True)
                            nc.tensor.matmul(o_ps1[:, cols],
                                             v16[:, vt, 128:256], pT[:],
                                             start=st, stop=sp_,
                                             skip_group_check=True)
                        last_in_blk = (kind == "diag" and u[4] == 3
                                       and u[5] == 4 * tqb + 3)
                        if last_in_blk:
                            qsl = slice(bT + tqb * 512, bT + (tqb + 1) * 512)
                            recip = rcp.tile([128, 512], F32, tag="rc")
                            nc.vector.reciprocal(recip[:], rb_ps[:])
                            # normalize (recip = OSCALE/rowsum) + hi/lo split
                            for half, o_ps in ((0, o_ps0), (1, o_ps1)):
                                g = 2 * h + half
                                on = rcp.tile([128, 512], F32, tag=f"on{half}")
                                nc.vector.tensor_mul(on[:], o_ps[:], recip[:])
                                ohi = OT8[:, g, 0, qsl]
                                nc.vector.tensor_copy(ohi, on[:])
                                ohif = rcp.tile([128, 512], F32,
                                                tag=f"ohif{half}")
                                nc.vector.tensor_copy(ohif[:], ohi)
                                nc.vector.tensor_sub(OT8[:, g, 1, qsl],
                                                     on[:], ohif[:])

            # ---------------- Phase D: output projection ----------------
            if "D" in phases:
                with ExitStack() as pd:
                    ysp = pd.enter_context(tc.tile_pool(name="ypool", bufs=2))
                    yps = pd.enter_context(
                        tc.tile_pool(name="ypsum", bufs=4, space="PSUM"))
                    for tt in range(T // 128):
                        ttsl = slice(tt * 128, (tt + 1) * 128)
                        # accumulate the full 4096-wide row in SBUF; one DMA
                        # per token tile keeps HWDGE off the critical path
                        y_sb = ysp.tile([128, d], BF16)
                        for eb in range(d // 512):
                            ebsl = slice(eb * 512, (eb + 1) * 512)
                            y_ps = yps.tile([128, 512], F32)
                            for gp in range(2):
                                nc.tensor.matmul(
                                    y_ps[:],
                                    OT8[:, 2 * gp:2 * gp + 2, 0, ttsl],
                                    wo_sb[:, 2 * gp:2 * gp + 2, 1, ebsl],
                                    start=(gp == 0), stop=False, perf_mode=DR)
                            for g in range(4):
                                nc.tensor.matmul(
                                    y_ps[:], OT8[:, g, :, ttsl],
                                    wo_sb[:, g, :, ebsl],
                                    start=False, stop=(g == 3), perf_mode=DR)
                            nc.scalar.mul(y_sb[:, ebsl], y_ps[:],
                                          1.0 / (WSCALE * OSCALE))
                            if tt >= T // 128 - 2:
                                # stream the last tiles per-block so the
                                # final transfers are small
                                nc.sync.dma_start(y[ttsl, ebsl],
                                                  y_sb[:, ebsl])
                        if tt < T // 128 - 2:
                            nc.sync.dma_start(y[ttsl, :], y_sb[:])

    if legalize:
        legalize_waits(nc)
    return nc


def _split8(a):
    hi = a.astype(E4)
    lo = (a - hi.astype(np.float32)).astype(E4)
    return hi, lo


def host_inputs(x, Wq, Wk, Wv, Wo, k_scale, v_scale, position, core,
                b=B, s=S, d=D):
    """Build the per-core input map."""
    T = b * s
    NF = d // 128
    NCH = T // 128

    xT = np.ascontiguousarray(x.reshape(T, d).T).astype(np.float32)
    xhi, xlo = _split8(xT)
    # xpack[ch, p, f, (hi,lo), t]
    xpack = np.empty((NCH, 128, NF, 2, 128), dtype=E4)
    xpack[:, :, :, 0, :] = xhi.reshape(NF, 128, NCH, 128).transpose(2, 1, 0, 3)
    xpack[:, :, :, 1, :] = xlo.reshape(NF, 128, NCH, 128).transpose(2, 1, 0, 3)

    def packw(w, nt):
        # w: [nt*128, width] -> [128, nt, (lo,hi), width]
        whi, wlo = _split8(w * WSCALE)
        out = np.empty((128, nt, 2, w.shape[1]), dtype=E4)
        out[:, :, 0, :] = wlo.reshape(nt, 128, -1).transpose(1, 0, 2)
        out[:, :, 1, :] = whi.reshape(nt, 128, -1).transpose(1, 0, 2)
        return out

    wq8 = packw(np.ascontiguousarray(Wq[:, core * DQ:(core + 1) * DQ]), NF)
    wkv = np.concatenate(
        [Wk[:, core * 256:(core + 1) * 256],
         Wv[:, core * 256:(core + 1) * 256]], axis=1)
    wkv8 = packw(np.ascontiguousarray(wkv), NF)
    wo8 = packw(np.ascontiguousarray(Wo[core * DQ:(core + 1) * DQ, :]), 4)

    pos = position.reshape(T).astype(np.float32)
    j = np.arange(128, dtype=np.float32)
    timescale = ROPE_BASE ** (2.0 * j / HD)
    ang = pos[:, None] / timescale[None, :]
    cosd = np.cos(ang).astype(np.float32)
    sind = np.sin(ang).astype(np.float32)

    ksc = np.broadcast_to((1.0 + k_scale).astype(np.float32), (128, 256)).copy()
    vsc = np.broadcast_to((1.0 + v_scale).astype(np.float32), (128, 256)).copy()
    ident = np.eye(128, dtype=np.float32)
    ones = np.full((128, 128), 1.0 / OSCALE, dtype=ml_dtypes.bfloat16)
    p = np.arange(128)[:, None]
    c = np.arange(128)[None, :]
    tri = np.where(p <= c, 1.0, 0.0).astype(ml_dtypes.bfloat16)

    return {
        "xpackd": xpack, "wq8d": wq8, "wkv8d": wkv8, "wo8d": wo8,
        "cosqd": np.ascontiguousarray((cosd / WSCALE).T).astype(np.float32),
        "sinqd": np.ascontiguousarray((sind / WSCALE).T).astype(np.float32),
        "coskd": cosd, "sinkd": sind,
        "kscd": ksc, "vscd": vsc, "identd": ident, "onesd": ones,
        "trid": tri,
    }


def kernel(x, Wq, Wk, Wv, Wo, k_scale, v_scale, mask, position):
    from concourse.bass_utils import run_bass_kernel_spmd
    b, s, d = x.shape
    nc = build_bass(b=b, s=s, d=d)
    in_maps = [
        host_inputs(x, Wq, Wk, Wv, Wo, k_scale, v_scale, position, core,
                    b=b, s=s, d=d)
        for core in range(N_CORES)
    ]
    res = run_bass_kernel_spmd(nc, in_maps, list(range(N_CORES)))
    out = None
    for r in res.results:
        yc = r["y"].astype(np.float32)
        out = yc if out is None else out + yc
    return out.reshape(b, s, d).astype(np.float32)


# revision 81
# speedup vs baseline: 1.0016x; 1.0016x over previous
"""Tensor-parallel MultiHeadAttention (GQA + RMSNorm-KV + RoPE) for 8 trn2 cores.

Sharding: KV head h -> core h (HKV=8); Q heads {2h, 2h+1}; x replicated;
Wo row-sharded; host sums the 8 partial outputs.

Matmuls use fp8e4 DoubleRow with a hi+lo split (3-term products) for the
Q/KV projections, attention scores, and the output projection; P (softmax
weights) and V stay bf16.  Weights are pre-scaled by 64 so the fp8 lo-term
stays out of the subnormal floor: the K/V RMSNorm cancels the scale
automatically, Q is descaled through pre-scaled RoPE tables, and the
output projection's 8*64 factor is divided out when copying y to SBUF
(with the 8 coming from ones=1/8 in the softmax denominator).
All intermediates (qT/kT/v/OT) are SBUF-resident - no DRAM round trips.
"""
import sys
sys.path.insert(0, '/opt/trn_rl_repo')
import numpy as np
import ml_dtypes
import concourse.bass as bass
import concourse.tile as tile
from concourse import mybir
from contextlib import ExitStack

F32 = mybir.dt.float32
F32R = mybir.dt.float32r
BF16 = mybir.dt.bfloat16
F8 = mybir.dt.float8e4
E4 = ml_dtypes.float8_e4m3
AF = mybir.ActivationFunctionType
DR = mybir.MatmulPerfMode.DoubleRow

# Problem constants (full size).
B = 2
S = 2048          # sequence per batch
D = 4096          # model dim
HD = 256          # head dim
DQ = 512          # per-core q width (2 heads)
ROPE_BASE = 10000.0
MASKV = -1e10
EPS = 1e-6
N_CORES = 8
WSCALE = 64.0     # weight pre-scale (power of 2; keeps fp8 lo-terms normal)
OSCALE = 8.0      # attention-output pre-scale via ones = 1/OSCALE


def legalize_waits(nc, max_waits=1):
    """This container's walrus encodes at most one sem-wait per instruction.
    Move extra waits onto same-engine NOPs placed just before (engine FIFO
    order makes that equivalent)."""
    n = 0
    for f in nc.m.functions:
        for blk in f.blocks:
            out = []
            for ins in blk.instructions:
                si = ins.sync_info
                if si is not None and si.on_wait and len(si.on_wait) > max_waits:
                    waits = list(si.on_wait)
                    for w in waits[max_waits:]:
                        nop = mybir.InstNoOp(name=nc.get_next_instruction_name())
                        nop.engine = ins.engine
                        nop.sync_info = mybir.SyncInfo(on_wait=[w], on_update=[])
                        out.append(nop)
                    ins.sync_info = mybir.SyncInfo(
                        on_wait=waits[:max_waits], on_update=list(si.on_update or []))
                    n += 1
                out.append(ins)
            blk.instructions.clear()
            for i in out:
                blk.instructions.append(i)
    return n


def build_bass(b=B, s=S, d=D, legalize=True, phases="ABCD"):
    T = b * s              # total tokens
    NF = d // 128          # contraction tiles
    CH = 128               # phase-A token chunk (x is stationary)
    NCH = T // CH
    TTB = s // 128         # token tiles per batch
    NQB = s // 512         # 512-wide query blocks per batch

    nc = bass.Bass()
    # packed fp8 inputs; i-axis order: x/OT are (hi, lo), weights are (lo, hi)
    xpackd = nc.dram_tensor("xpackd", [NCH, 128, NF, 2, CH], F8,
                            kind="ExternalInput")
    wq8d = nc.dram_tensor("wq8d", [128, NF, 2, DQ], F8, kind="ExternalInput")
    wkv8d = nc.dram_tensor("wkv8d", [128, NF, 2, 512], F8,
                           kind="ExternalInput")
    wo8d = nc.dram_tensor("wo8d", [128, 4, 2, d], F8, kind="ExternalInput")
    cosqd = nc.dram_tensor("cosqd", [128, T], F32, kind="ExternalInput")
    sinqd = nc.dram_tensor("sinqd", [128, T], F32, kind="ExternalInput")
    coskd = nc.dram_tensor("coskd", [T, 128], F32, kind="ExternalInput")
    sinkd = nc.dram_tensor("sinkd", [T, 128], F32, kind="ExternalInput")
    kscd = nc.dram_tensor("kscd", [128, 256], F32, kind="ExternalInput")
    vscd = nc.dram_tensor("vscd", [128, 256], F32, kind="ExternalInput")
    identd = nc.dram_tensor("identd", [128, 128], F32R, kind="ExternalInput")
    onesd = nc.dram_tensor("onesd", [128, 128], BF16, kind="ExternalInput")
    trid = nc.dram_tensor("trid", [128, 128], BF16, kind="ExternalInput")
    y = nc.dram_tensor("y", [T, d], BF16, kind="ExternalOutput")

    with tile.TileContext(nc) as tc, ExitStack() as top:
        # const tiles allocated up front; DMAs emitted later so the first x
        # chunk + weight split win the (serialized) DMA-engine queue
        cp = top.enter_context(tc.tile_pool(name="const", bufs=1))
        zero_b = cp.tile([128, 1], F32)
        nc.vector.memset(zero_b[:], 0.0)
        eps_b = cp.tile([128, 1], F32)
        nc.vector.memset(eps_b[:], EPS)
        ksc = cp.tile([128, 256], F32)
        vsc = cp.tile([128, 256], F32)
        ident = cp.tile([128, 128], F32R)
        ones_sb = cp.tile([128, 128], BF16)
        trib = cp.tile([128, 128], BF16)

        # SBUF-resident intermediates
        ip = top.enter_context(tc.tile_pool(name="interm", bufs=1))
        # qT8[p, head, hd-tile, (lo,hi), T]
        qT8 = ip.tile([128, 2, 2, 2, T], F8)
        # kT8[p, hd-tile, (hi,lo), T]
        kT8 = ip.tile([128, 2, 2, T], F8)
        # v16[p, token-tile, hd]
        v16 = ip.tile([128, NCH, 256], BF16)

        # ---------------- Phase A: projections + norm + rope ----------------
        with ExitStack() as pa:
            wp = pa.enter_context(tc.tile_pool(name="wpool", bufs=1))
            xp = pa.enter_context(tc.tile_pool(name="xpool", bufs=3))
            ep = pa.enter_context(tc.tile_pool(name="aeps", bufs=2))
            pp = pa.enter_context(tc.tile_pool(name="apsum", bufs=2, space="PSUM"))
            tp = pa.enter_context(tc.tile_pool(name="atpsum", bufs=2, space="PSUM"))

            wq_sb = wp.tile([128, NF, 2, DQ], F8)
            wkv_sb = wp.tile([128, NF, 2, 512], F8)
            SPLITS = [0, 2, 12, 22, 32]   # first split tiny: the
            # first chunk's matmuls start after just x0 + 2 f-tiles of weights
            NSPLIT = len(SPLITS) - 1

            x_tiles = {}

            def emit_x_dma(ch):
                x_sb = xp.tile([128, NF, 2, CH], F8, tag="x")
                nc.sync.dma_start(x_sb[:], xpackd[ch])
                x_tiles[ch] = x_sb
                return x_sb

            def alloc_x(ch):
                x_sb = xp.tile([128, NF, 2, CH], F8, tag="x")
                x_tiles[ch] = x_sb
                return x_sb

            def emit_x_piece(x_sb, ch, sp):
                f0, f1 = SPLITS[sp], SPLITS[sp + 1]
                nc.sync.dma_start(x_sb[:, f0:f1], xpackd[ch, :, f0:f1])

            # startup: the first three chunks' x arrives in split-aligned
            # pieces interleaved with the weight splits, so each arriving
            # weight split already has matching x data to compute on
            xs0, xs1, xs2 = alloc_x(0), alloc_x(1), alloc_x(2)
            emit_x_piece(xs0, 0, 0)
            for sp in range(NSPLIT):
                f0, f1 = SPLITS[sp], SPLITS[sp + 1]
                nc.sync.dma_start(wq_sb[:, f0:f1], wq8d[:, f0:f1])
                nc.sync.dma_start(wkv_sb[:, f0:f1], wkv8d[:, f0:f1])
                if sp + 1 < NSPLIT:
                    emit_x_piece(xs0, 0, sp + 1)
                emit_x_piece(xs1, 1, sp)
                if sp >= 1:
                    emit_x_piece(xs2, 2, sp - 1)
                if sp == 0:
                    nc.sync.dma_start(ksc[:], kscd[:])
                    nc.sync.dma_start(vsc[:], vscd[:])
                    nc.sync.dma_start(ident[:], identd[:])
            emit_x_piece(xs2, 2, NSPLIT - 1)

            def emit_kv_split(x_sb, ps_kv, sp):
                # kv: x stationary -> ps_kv[token, width]
                f0, f1 = SPLITS[sp], SPLITS[sp + 1]
                for fp_ in range(f0 // 2, f1 // 2):
                    lhs = x_sb[:, 2 * fp_:2 * fp_ + 2, 0, :]
                    nc.tensor.matmul(ps_kv[:], lhs,
                                     wkv_sb[:, 2 * fp_:2 * fp_ + 2, 1, :],
                                     start=(fp_ == 0), stop=False,
                                     perf_mode=DR)
                for f in range(f0, f1):
                    lhs = x_sb[:, f, :, :]
                    nc.tensor.matmul(ps_kv[:], lhs, wkv_sb[:, f, :, :],
                                     start=False, stop=(f == NF - 1),
                                     perf_mode=DR)

            def emit_q_split(x_sb, ps_q, m, sp):
                # q: WEIGHTS stationary, x moving -> ps_q[:, m, :] =
                # q^T[hd-tile m, tokens] directly (no PE transpose later).
                # start=True zeroes the whole 2KB bank, so each m-chain must
                # run to completion before the next one starts
                f0, f1 = SPLITS[sp], SPLITS[sp + 1]
                msl = slice(m * 128, (m + 1) * 128)
                for fp_ in range(f0 // 2, f1 // 2):
                    nc.tensor.matmul(
                        ps_q[:, m, :],
                        wq_sb[:, 2 * fp_:2 * fp_ + 2, 1, msl],
                        x_sb[:, 2 * fp_:2 * fp_ + 2, 0, :],
                        start=(m == 0 and fp_ == 0), stop=False,
                        perf_mode=DR, skip_group_check=True)
                for f in range(f0, f1):
                    nc.tensor.matmul(
                        ps_q[:, m, :], wq_sb[:, f, :, msl], x_sb[:, f, :, :],
                        start=False, stop=(m == 3 and f == NF - 1),
                        perf_mode=DR, skip_group_check=True)

            def get_x(ch):
                x_sb = x_tiles.pop(ch) if ch in x_tiles else emit_x_dma(ch)
                x_tiles.pop(ch, None)
                return x_sb

            def emit_chunk_mms(ch):
                x_sb = get_x(ch)
                ps_q = pp.tile([128, 4, 128], F32, tag="psq")
                ps_kv = pp.tile([128, 512], F32, tag="pskv")
                # grouped by weight DMA split so the first chunks can start
                # before all weight splits have landed
                for sp in range(NSPLIT):
                    emit_kv_split(x_sb, ps_kv, sp)
                for m in range(4):
                    for sp in range(NSPLIT):
                        emit_q_split(x_sb, ps_q, m, sp)
                return ps_q, ps_kv

            def emit_first_two_chunk_mms():
                # interleave chunks 0 and 1 split-by-split: each arriving
                # weight split immediately has two chunks of ready PE work
                xs = [get_x(0), get_x(1)]
                pss = []
                for c in range(2):
                    ps_q = pp.tile([128, 4, 128], F32, tag="psq")
                    ps_kv = pp.tile([128, 512], F32, tag="pskv")
                    pss.append((ps_q, ps_kv))
                for sp in range(NSPLIT):
                    for c in range(2):
                        emit_kv_split(xs[c], pss[c][1], sp)
                for m in range(4):
                    for sp in range(NSPLIT):
                        for c in range(2):
                            emit_q_split(xs[c], pss[c][0], m, sp)
                return pss

            def emit_chunk_dve(ch, ps_q, ps_kv):
                tg = ch * CH
                cosq_t = ep.tile([128, 128], F32, tag="cosq")
                nc.sync.dma_start(cosq_t[:], cosqd[:, tg:tg + 128])
                sinq_t = ep.tile([128, 128], F32, tag="sinq")
                nc.sync.dma_start(sinq_t[:], sinqd[:, tg:tg + 128])
                cosk_t = ep.tile([128, 128], F32, tag="cosk")
                nc.sync.dma_start(cosk_t[:], coskd[tg:tg + 128, :])
                sink_t = ep.tile([128, 128], F32, tag="sink")
                nc.sync.dma_start(sink_t[:], sinkd[tg:tg + 128, :])

                # Q rope in q^T layout (cos/sin tables are [hd-part, T]
                # with the rope frequency = partition); cosq/sinq carry the
                # 1/WSCALE descale
                stage_q = ep.tile([128, 2, 2, 128], F32, tag="stq")
                tmp1 = ep.tile([128, 128], F32, tag="tmp1")
                tmp2 = ep.tile([128, 128], F32, tag="tmp2")
                for h in range(2):
                    fi = ps_q[:, 2 * h, :]
                    se = ps_q[:, 2 * h + 1, :]
                    nc.vector.tensor_mul(tmp1[:], fi, cosq_t[:])
                    nc.vector.tensor_mul(tmp2[:], se, sinq_t[:])
                    nc.vector.tensor_sub(stage_q[:, h, 0, :],
                                         tmp1[:], tmp2[:])
                    nc.vector.tensor_mul(tmp1[:], se, cosq_t[:])
                    nc.vector.tensor_mul(tmp2[:], fi, sinq_t[:])
                    nc.vector.tensor_add(stage_q[:, h, 1, :],
                                         tmp1[:], tmp2[:])
                # split q into qT8 right here -- already transposed
                qhi = qT8[:, :, :, 1, tg:tg + 128]
                nc.vector.tensor_copy(qhi, stage_q[:])
                qhif = ep.tile([128, 2, 2, 128], F32, tag="qhif")
                nc.vector.tensor_copy(qhif[:], qhi)
                nc.vector.tensor_sub(qT8[:, :, :, 0, tg:tg + 128],
                                     stage_q[:], qhif[:])

                # K: rmsnorm + scale + rope (scale-invariant in WSCALE)
                sq = ep.tile([128, 256], F32, tag="sq")
                ssq = ep.tile([128, 1], F32, tag="ssq")
                nc.scalar.activation(sq[:], ps_kv[:, 0:256], AF.Square,
                                     bias=zero_b[:], accum_out=ssq[:])
                std = ep.tile([128, 1], F32, tag="std")
                nc.scalar.activation(std[:], ssq[:], AF.Sqrt,
                                     bias=eps_b[:], scale=1.0 / 256.0)
                rstd = ep.tile([128, 1], F32, tag="rstd")
                nc.vector.reciprocal(rstd[:], std[:])
                kn = ep.tile([128, 256], F32, tag="kn")
                nc.vector.tensor_scalar_mul(kn[:], ps_kv[:, 0:256], rstd[:])
                kn2 = ep.tile([128, 256], F32, tag="kn2")
                nc.vector.tensor_mul(kn2[:], kn[:], ksc[:])
                stage_k = ep.tile([128, 256], F32R, tag="stk")
                nc.vector.tensor_mul(tmp1[:], kn2[:, 0:128], cosk_t[:])
                nc.vector.tensor_mul(tmp2[:], kn2[:, 128:256], sink_t[:])
                nc.vector.tensor_sub(stage_k[:, 0:128], tmp1[:], tmp2[:])
                nc.vector.tensor_mul(tmp1[:], kn2[:, 128:256], cosk_t[:])
                nc.vector.tensor_mul(tmp2[:], kn2[:, 0:128], sink_t[:])
                nc.vector.tensor_add(stage_k[:, 128:256], tmp1[:], tmp2[:])

                # V: rmsnorm + scale -> straight into SBUF-resident v16 (bf16)
                sqv = ep.tile([128, 256], F32, tag="sqv")
                ssqv = ep.tile([128, 1], F32, tag="ssqv")
                nc.scalar.activation(sqv[:], ps_kv[:, 256:512], AF.Square,
                                     bias=zero_b[:], accum_out=ssqv[:])
                stdv = ep.tile([128, 1], F32, tag="stdv")
                nc.scalar.activation(stdv[:], ssqv[:], AF.Sqrt,
                                     bias=eps_b[:], scale=1.0 / 256.0)
                rstdv = ep.tile([128, 1], F32, tag="rstdv")
                nc.vector.reciprocal(rstdv[:], stdv[:])
                vn = ep.tile([128, 256], F32, tag="vn")
                nc.vector.tensor_scalar_mul(vn[:], ps_kv[:, 256:512], rstdv[:])
                nc.vector.tensor_mul(v16[:, ch, :], vn[:], vsc[:])
                return stage_k

            def emit_chunk_transposes(ch, stage_k):
                tg = ch * CH
                tk_ps = tp.tile([128, 256], F32, tag="tk")
                for dblk in range(2):
                    nc.tensor.matmul(
                        tk_ps[:, dblk * 128:(dblk + 1) * 128].bitcast(F32R),
                        stage_k[:, dblk * 128:(dblk + 1) * 128],
                        ident[:], is_transpose=True)
                khi = kT8[:, :, 0, tg:tg + 128]
                nc.vector.tensor_copy(khi, tk_ps[:])
                khif = ep.tile([128, 256], F32, tag="khif")
                nc.vector.tensor_copy(khif[:], khi)
                nc.vector.tensor_sub(kT8[:, :, 1, tg:tg + 128],
                                     tk_ps[:], khif[:])

            pending = None
            for ch in (range(NCH) if "A" in phases else []):
                ps_q, ps_kv = emit_chunk_mms(ch)
                stage_k = emit_chunk_dve(ch, ps_q, ps_kv)
                if pending is not None:
                    emit_chunk_transposes(pending[0], pending[1])
                pending = (ch, stage_k)
            if pending is not None:
                emit_chunk_transposes(pending[0], pending[1])

        # ---------------- Phases B/C: attention ----------------
        with ExitStack() as pot:
            otp = pot.enter_context(tc.tile_pool(name="otpool", bufs=1))
            # OT8[p, g=(head,hd-half), (hi,lo), T], pre-scaled by OSCALE
            OT8 = otp.tile([128, 4, 2, T], F8)
            wo_sb = otp.tile([128, 4, 2, d], F8)
            nc.sync.dma_start(ones_sb[:], onesd[:])
            nc.sync.dma_start(trib[:], trid[:])
            for g in range(4):
                nc.sync.dma_start(wo_sb[:, g], wo8d[:, g])

            if "B" in phases and "C" in phases:
                with ExitStack() as pc:
                    spool = pc.enter_context(
                        tc.tile_pool(name="spsum", bufs=3, space="PSUM"))
                    opool = pc.enter_context(
                        tc.tile_pool(name="opsum", bufs=2, space="PSUM"))
                    rpool = pc.enter_context(
                        tc.tile_pool(name="rpsum", bufs=1, space="PSUM"))
                    ptp = pc.enter_context(tc.tile_pool(name="ptpool", bufs=14))
                    rcp = pc.enter_context(tc.tile_pool(name="rcpool", bufs=2))

                    # unit = ("full", bb, h, tqb, j): unmasked [128k x 512q]
                    # unit = ("diag", bb, h, tqb, qs, jd): [128k x 128q] on
                    #   the causal diagonal; masked triangularly when
                    #   jd == 4*tqb + qs, else unmasked
                    units = []
                    for bb in range(b):
                        for h in range(2):
                            for tqb in range(NQB):
                                for j in range(4 * tqb):
                                    units.append(("full", bb, h, tqb, j))
                                for qs in range(4):
                                    for jd in range(4 * tqb,
                                                    4 * tqb + qs + 1):
                                        units.append(
                                            ("diag", bb, h, tqb, qs, jd))

                    # diag score blocks pack 4-per-PSUM-tile (column slots) so
                    # a deep lookahead fits in 3 score banks
                    dstate = {"tile": None, "slot": 3}

                    def emit_s(u):
                        kind, bb, h, tqb = u[:4]
                        bT = bb * s
                        if kind == "full":
                            j = u[4]
                            qsl = slice(bT + tqb * 512, bT + (tqb + 1) * 512)
                            jsl = slice(bT + j * 128, bT + (j + 1) * 128)
                            s_ps = spool.tile([128, 512], F32, tag="s",
                                              name="s_ps")
                            out = s_ps[:]
                            dstate["tile"] = None
                        else:
                            qs, jd = u[4], u[5]
                            q0 = bT + tqb * 512 + qs * 128
                            qsl = slice(q0, q0 + 128)
                            jsl = slice(bT + jd * 128, bT + (jd + 1) * 128)
                            slot = dstate["slot"] + 1
                            if slot > 3 or dstate["tile"] is None:
                                dstate["tile"] = spool.tile(
                                    [128, 512], F32, tag="s", name="s_ps")
                                slot = 0
                            dstate["slot"] = slot
                            out = dstate["tile"][:, slot * 128:slot * 128 + 128]
                        nc.tensor.matmul(
                            out, kT8[:, :, 0, jsl], qT8[:, h, :, 1, qsl],
                            start=True, stop=False, perf_mode=DR)
                        nc.tensor.matmul(
                            out, kT8[:, 0, :, jsl], qT8[:, h, 0, :, qsl],
                            start=False, stop=False, perf_mode=DR)
                        nc.tensor.matmul(
                            out, kT8[:, 1, :, jsl], qT8[:, h, 1, :, qsl],
                            start=False, stop=True, perf_mode=DR)
                        return out

                    # one software pipeline over all units with a deep score
                    # lookahead: the PE never waits on the scalar engine
                    # across block edges
                    LOOK = 14
                    o_ps0 = o_ps1 = rb_ps = None
                    s_q = [emit_s(uu) for uu in units[:LOOK]]
                    prev_blk = None
                    for i, u in enumerate(units):
                        kind, bb, h, tqb = u[:4]
                        bT = bb * s
                        blk = (bb, h, tqb)
                        s_cur = s_q.pop(0)
                        if kind == "full":
                            pT = ptp.tile([128, 512], BF16, tag="pf")
                            nc.scalar.activation(pT[:], s_cur, AF.Exp,
                                                 bias=zero_b[:], scale=0.0625)
                        else:
                            pT = ptp.tile([128, 128], BF16, tag="pd")
                            nc.scalar.activation(pT[:], s_cur,
                                                 AF.Exp, bias=zero_b[:],
                                                 scale=0.0625)
                            if u[5] == 4 * tqb + u[4]:
                                # causal mask: zero the upper triangle of the
                                # diagonal block on the otherwise-idle Pool
                                # engine (exp of unmasked scores is finite)
                                nc.gpsimd.tensor_mul(pT[:], pT[:], trib[:])
                        if i + LOOK < len(units):
                            s_q.append(emit_s(units[i + LOOK]))
                        if blk != prev_blk:
                            o_ps0 = opool.tile([128, 512], F32, tag="o0")
                            o_ps1 = opool.tile([128, 512], F32, tag="o1")
                            rb_ps = rpool.tile([128, 512], F32, tag="rb")
                            prev_blk = blk
                        if kind == "full":
                            j = u[4]
                            vt = bb * TTB + j
                            st = (j == 0)
                            nc.tensor.matmul(rb_ps[:], ones_sb[:], pT[:],
                                             start=st, stop=False,
                                             skip_group_check=True)
                            nc.tensor.matmul(o_ps0[:], v16[:, vt, 0:128],
                                             pT[:], start=st, stop=False,
                                             skip_group_check=True)
                            nc.tensor.matmul(o_ps1[:], v16[:, vt, 128:256],
                                             pT[:], start=st, stop=False,
                                             skip_group_check=True)
                        else:
                            qs, jd = u[4], u[5]
                            vt = bb * TTB + jd
                            cols = slice(qs * 128, (qs + 1) * 128)
                            st = (tqb == 0 and jd == 0)
                            sp_ = (jd == 4 * tqb + qs)
                            nc.tensor.matmul(rb_ps[:, cols], ones_sb[:],
                                             pT[:], start=st, stop=sp_,
                                             skip_group_check=True)
                            nc.tensor.matmul(o_ps0[:, cols],
                                             v16[:, vt, 0:128], pT[:],
                                             start=st, stop=sp_,
                                             skip_group_check=True)
                            nc.tensor.matmul(o_ps1[:, cols],
                                             v16[:, vt, 128:256], pT[:],
                                             start=st, stop=sp_,
                                             skip_group_check=True)
                        last_in_blk = (kind == "diag" and u[4] == 3
                                       and u[5] == 4 * tqb + 3)
                        if last_in_blk:
                            qsl = slice(bT + tqb * 512, bT + (tqb + 1) * 512)
                            recip = rcp.tile([128, 512], F32, tag="rc")
                            nc.vector.reciprocal(recip[:], rb_ps[:])
                            # normalize (recip = OSCALE/rowsum) + hi/lo split
                            for half, o_ps in ((0, o_ps0), (1, o_ps1)):
                                g = 2 * h + half
                                on = rcp.tile([128, 512], F32, tag=f"on{half}")
                                nc.vector.tensor_mul(on[:], o_ps[:], recip[:])
                                ohi = OT8[:, g, 0, qsl]
                                nc.vector.tensor_copy(ohi, on[:])
                                ohif = rcp.tile([128, 512], F32,
                                                tag=f"ohif{half}")
                                nc.vector.tensor_copy(ohif[:], ohi)
                                nc.vector.tensor_sub(OT8[:, g, 1, qsl],
                                                     on[:], ohif[:])

            # ---------------- Phase D: output projection ----------------
            if "D" in phases:
                with ExitStack() as pd:
                    ysp = pd.enter_context(tc.tile_pool(name="ypool", bufs=2))
                    yps = pd.enter_context(
                        tc.tile_pool(name="ypsum", bufs=4, space="PSUM"))
                    for tt in range(T // 128):
                        ttsl = slice(tt * 128, (tt + 1) * 128)
                        # accumulate the full 4096-wide row in SBUF; one DMA
                        # per token tile keeps HWDGE off the critical path
                        y_sb = ysp.tile([128, d], BF16)
                        for eb in range(d // 512):
                            ebsl = slice(eb * 512, (eb + 1) * 512)
                            y_ps = yps.tile([128, 512], F32)
                            for gp in range(2):
                                nc.tensor.matmul(
                                    y_ps[:],
                                    OT8[:, 2 * gp:2 * gp + 2, 0, ttsl],
                                    wo_sb[:, 2 * gp:2 * gp + 2, 1, ebsl],
                                    start=(gp == 0), stop=False, perf_mode=DR)
                            for g in range(4):
                                nc.tensor.matmul(
                                    y_ps[:], OT8[:, g, :, ttsl],
                                    wo_sb[:, g, :, ebsl],
                                    start=False, stop=(g == 3), perf_mode=DR)
                            nc.scalar.mul(y_sb[:, ebsl], y_ps[:],
                                          1.0 / (WSCALE * OSCALE))
                            if tt >= T // 128 - 2:
                                # stream the last tiles per-block so the
                                # final transfers are small
                                nc.sync.dma_start(y[ttsl, ebsl],
                                                  y_sb[:, ebsl])
                        if tt < T // 128 - 2:
                            nc.sync.dma_start(y[ttsl, :], y_sb[:])

    if legalize:
        legalize_waits(nc)
    return nc


def _split8(a):
    hi = a.astype(E4)
    lo = (a - hi.astype(np.float32)).astype(E4)
    return hi, lo


def host_inputs(x, Wq, Wk, Wv, Wo, k_scale, v_scale, position, core,
                b=B, s=S, d=D):
    """Build the per-core input map."""
    T = b * s
    NF = d // 128
    NCH = T // 128

    xT = np.ascontiguousarray(x.reshape(T, d).T).astype(np.float32)
    xhi, xlo = _split8(xT)
    # xpack[ch, p, f, (hi,lo), t]
    xpack = np.empty((NCH, 128, NF, 2, 128), dtype=E4)
    xpack[:, :, :, 0, :] = xhi.reshape(NF, 128, NCH, 128).transpose(2, 1, 0, 3)
    xpack[:, :, :, 1, :] = xlo.reshape(NF, 128, NCH, 128).transpose(2, 1, 0, 3)

    def packw(w, nt):
        # w: [nt*128, width] -> [128, nt, (lo,hi), width]
        whi, wlo = _split8(w * WSCALE)
        out = np.empty((128, nt, 2, w.shape[1]), dtype=E4)
        out[:, :, 0, :] = wlo.reshape(nt, 128, -1).transpose(1, 0, 2)
        out[:, :, 1, :] = whi.reshape(nt, 128, -1).transpose(1, 0, 2)
        return out

    wq8 = packw(np.ascontiguousarray(Wq[:, core * DQ:(core + 1) * DQ]), NF)
    wkv = np.concatenate(
        [Wk[:, core * 256:(core + 1) * 256],
         Wv[:, core * 256:(core + 1) * 256]], axis=1)
    wkv8 = packw(np.ascontiguousarray(wkv), NF)
    wo8 = packw(np.ascontiguousarray(Wo[core * DQ:(core + 1) * DQ, :]), 4)

    pos = position.reshape(T).astype(np.float32)
    j = np.arange(128, dtype=np.float32)
    timescale = ROPE_BASE ** (2.0 * j / HD)
    ang = pos[:, None] / timescale[None, :]
    cosd = np.cos(ang).astype(np.float32)
    sind = np.sin(ang).astype(np.float32)

    ksc = np.broadcast_to((1.0 + k_scale).astype(np.float32), (128, 256)).copy()
    vsc = np.broadcast_to((1.0 + v_scale).astype(np.float32), (128, 256)).copy()
    ident = np.eye(128, dtype=np.float32)
    ones = np.full((128, 128), 1.0 / OSCALE, dtype=ml_dtypes.bfloat16)
    p = np.arange(128)[:, None]
    c = np.arange(128)[None, :]
    tri = np.where(p <= c, 1.0, 0.0).astype(ml_dtypes.bfloat16)

    return {
        "xpackd": xpack, "wq8d": wq8, "wkv8d": wkv8, "wo8d": wo8,
        "cosqd": np.ascontiguousarray((cosd / WSCALE).T).astype(np.float32),
        "sinqd": np.ascontiguousarray((sind / WSCALE).T).astype(np.float32),
        "coskd": cosd, "sinkd": sind,
        "kscd": ksc, "vscd": vsc, "identd": ident, "onesd": ones,
        "trid": tri,
    }


def kernel(x, Wq, Wk, Wv, Wo, k_scale, v_scale, mask, position):
    from concourse.bass_utils import run_bass_kernel_spmd
    b, s, d = x.shape
    nc = build_bass(b=b, s=s, d=d)
    in_maps = [
        host_inputs(x, Wq, Wk, Wv, Wo, k_scale, v_scale, position, core,
                    b=b, s=s, d=d)
        for core in range(N_CORES)
    ]
    res = run_bass_kernel_spmd(nc, in_maps, list(range(N_CORES)))
    out = None
    for r in res.results:
        yc = r["y"].astype(np.float32)
        out = yc if out is None else out + yc
    return out.reshape(b, s, d).astype(np.float32)
